# revision 43
# baseline (speedup 1.0000x reference)
"""Causal self-attention (B=4, T=2048, C=1024, H=16) on 8 TRN2 NeuronCores.

Sharding: core c handles batch b=c//2 and head-half hh=c%2 (8 heads).
Each core computes q/k/v projections for its heads, causal attention, and a
partial output projection (row-parallel w_proj); the host sums the two
partials per batch and adds b_proj plus the v-bias term (bv @ wp commutes
through the softmax average, so it is a host-side constant).

Layout: transposed attention (S^T = K Q^T with keys on psum partitions) so
softmax needs no transposes; a ones column in V yields softmax denominators
from the attn@v matmul. All matmul operands are bf16 (fp32 psum accumulate).
RoPE channel pairs are interleaved on partitions host-side so rotate-half is
a single DVE stream_shuffle (scores are invariant to the permutation).
x^T stays resident in SBUF. q/k/out projections run as K=64 row-group pairs
at tile_position (0,0)/(64,0) so the two streams overlap and weight loads
hide under the concurrent matmul. Denominator reciprocals run on a
DMA-transposed [128, chunk] layout (free-size-bound DVE op). Projections,
attention, and the output projection are software-pipelined per 512-query
chunk via a deferred work queue.
"""

import sys

sys.path.insert(0, "/opt/trn_rl_repo")

from contextlib import ExitStack

import numpy as np
import ml_dtypes

import concourse.bass as bass
import concourse.tile as tile
from concourse import bacc, mybir
from concourse.bass_utils import run_bass_kernel_spmd

F32 = mybir.dt.float32
BF16 = mybir.dt.bfloat16
AL = mybir.AluOpType
AF = mybir.ActivationFunctionType

B, T, C, H, HD = 4, 2048, 1024, 16, 64
NCORE = 8
HH = H // 2  # heads per core (8)
NP = HH // 2  # head pairs per core (4)
KC = C // 128  # contraction chunks (8)
NT = T // 128  # 128-row time tiles (16)
NQC = T // 512  # 512-query chunks (4)
ROPE_THETA = 10000.0
SHUF_SWAP = [l ^ 1 for l in range(32)]

_CACHE = {}


def _build_module():
    nc = bacc.Bacc("TRN2", target_bir_lowering=False, debug=False)

    xT = nc.dram_tensor("xT", [C, T], BF16, kind="ExternalInput")
    wq = nc.dram_tensor("wq", [C, 512], BF16, kind="ExternalInput")
    wk = nc.dram_tensor("wk", [C, 512], BF16, kind="ExternalInput")
    wv = nc.dram_tensor("wv", [C, 512], BF16, kind="ExternalInput")
    wp = nc.dram_tensor("wp", [512, C], BF16, kind="ExternalInput")
    bqk = nc.dram_tensor("bqk", [2, NP, 128], F32, kind="ExternalInput")
    cosr = nc.dram_tensor("cosr", [128, T], BF16, kind="ExternalInput")
    sinp = nc.dram_tensor("sinp", [128, T], BF16, kind="ExternalInput")
    mneg = nc.dram_tensor("mneg", [128, 2, 128], BF16, kind="ExternalInput")
    y = nc.dram_tensor("y", [T, C], F32, kind="ExternalOutput")

    with tile.TileContext(nc) as tc, ExitStack() as ctx:
        consts = ctx.enter_context(tc.tile_pool(name="consts", bufs=1))
        bigp = ctx.enter_context(tc.tile_pool(name="big", bufs=1))
        stg = ctx.enter_context(tc.tile_pool(name="stg", bufs=4))
        ptp = ctx.enter_context(tc.tile_pool(name="ptp", bufs=6))
        ocp = ctx.enter_context(tc.tile_pool(name="ocp", bufs=4))
        nrm = ctx.enter_context(tc.tile_pool(name="nrm", bufs=2))
        ysp = ctx.enter_context(tc.tile_pool(name="ysp", bufs=3))
        psg = ctx.enter_context(tc.tile_pool(name="psg", bufs=2, space="PSUM"))
        pss = ctx.enter_context(tc.tile_pool(name="pss", bufs=2, space="PSUM"))
        pso = ctx.enter_context(tc.tile_pool(name="pso", bufs=1, space="PSUM"))

        # ---- constants ----
        bqk_sb = consts.tile([128, 2, NP], F32)
        nc.sync.dma_start(out=bqk_sb[:], in_=bqk.rearrange("a p r -> r a p"))
        cos_sb = consts.tile([128, T], BF16)
        sin_sb = consts.tile([128, T], BF16)
        nc.sync.dma_start(out=cos_sb[:], in_=cosr[:])
        nc.sync.dma_start(out=sin_sb[:], in_=sinp[:])
        mask_sb = consts.tile([128, 2, 128], BF16)
        nc.sync.dma_start(out=mask_sb[:], in_=mneg[:])

        wq_sb = consts.tile([128, KC, 512], BF16)
        wk_sb = consts.tile([128, KC, 512], BF16)
        wv_sb = consts.tile([128, KC, 512], BF16)
        for wt, wsb in ((wq, wq_sb), (wk, wk_sb), (wv, wv_sb)):
            nc.sync.dma_start(
                out=wsb[:], in_=wt.rearrange("(kc p) m -> p kc m", p=128)
            )
        wp_sb = consts.tile([128, NP, C], BF16)
        nc.sync.dma_start(
            out=wp_sb[:], in_=wp.rearrange("(kc r) n -> r kc n", r=128)
        )

        # resident x^T, loaded once in 4 chunks so chunk-0 work starts early
        xT_sb = bigp.tile([128, KC, T], BF16)
        xTr = xT.rearrange("(kc p) t -> p kc t", p=128)
        for j in range(NQC):
            nk = slice(j * 512, (j + 1) * 512)
            nc.sync.dma_start(out=xT_sb[:, :, nk], in_=xTr[:, :, nk])

        qT_sb = bigp.tile([128, NP, T], BF16)
        kT_sb = bigp.tile([128, NP, T], BF16)
        vp_sb = bigp.tile([128, NT, HH, 65], BF16)
        OT_sb = bigp.tile([128, NP, T], BF16)
        nc.vector.memset(vp_sb[:, :, :, 64:65], 1.0)

        # ---- work units (closures), interleaved via a deferred queue ----
        def proj_v(tt):
            def go():
                vps = psg.tile([128, 512], F32, tag="g")
                for kc in range(KC):
                    nc.tensor.matmul(
                        vps[:],
                        xT_sb[:, kc, tt * 128 : (tt + 1) * 128],
                        wv_sb[:, kc, :],
                        start=(kc == 0),
                        stop=(kc == KC - 1),
                    )
                nc.scalar.activation(
                    vp_sb[:, tt, :, 0:64],
                    vps.rearrange("p (h d) -> p h d", h=HH),
                    AF.Identity,
                )

            return go

        def proj_qk(which, wsb, qkout, nq, p):
            def go():
                nk = slice(nq * 512, (nq + 1) * 512)
                qps = psg.tile([128, 512], F32, tag="g")
                for kc in range(KC):
                    nc.tensor.matmul(
                        qps[:],
                        wsb[:, kc, p * 128 : (p + 1) * 128],
                        xT_sb[:, kc, nk],
                        start=(kc == 0),
                        stop=(kc == KC - 1),
                    )
                # evict psum + per-partition bias on ScalarE (ACT is idle
                # outside exp; keeps DVE free for attention evictions)
                qsf = stg.tile([128, 512], BF16, tag="qsf")
                nc.scalar.activation(
                    qsf[:], qps[:], AF.Identity, bias=bqk_sb[:, which, p : p + 1]
                )
                qsh = stg.tile([128, 512], BF16, tag="qsh")
                nc.vector.stream_shuffle(qsh[:], qsf[:], SHUF_SWAP)
                m1 = stg.tile([128, 512], BF16, tag="m1")
                nc.vector.tensor_mul(m1[:], qsf[:], cos_sb[:, nk])
                m2 = stg.tile([128, 512], BF16, tag="m2")
                nc.vector.tensor_mul(m2[:], qsh[:], sin_sb[:, nk])
                nc.vector.tensor_add(qkout[:, p, nk], m1[:], m2[:])

            return go

        def out_proj(tt):
            def go():
                ysb = ysp.tile([128, C], F32, tag="ysb")
                for nn in range(2):
                    yps = psg.tile([128, 512], F32, tag="g")
                    for kc in range(NP):
                        nc.tensor.matmul(
                            yps[:],
                            OT_sb[:, kc, tt * 128 : (tt + 1) * 128],
                            wp_sb[:, kc, nn * 512 : (nn + 1) * 512],
                            start=(kc == 0),
                            stop=(kc == NP - 1),
                        )
                    nc.scalar.activation(
                        ysb[:, nn * 512 : (nn + 1) * 512], yps[:], AF.Identity
                    )
                nc.sync.dma_start(out=y[tt * 128 : (tt + 1) * 128, :], in_=ysb[:])

            return go

        work_q = []

        def pop_work(n):
            for _ in range(min(n, len(work_q))):
                work_q.pop(0)()

        def queue_proj(j):
            for tt in range(j * 4, (j + 1) * 4):
                work_q.append(proj_v(tt))
            for p in range(NP):
                work_q.append(proj_qk(0, wq_sb, qT_sb, j, p))
                work_q.append(proj_qk(1, wk_sb, kT_sb, j, p))

        # chunk-0 projections must run before attention j=0
        queue_proj(0)
        pop_work(len(work_q))

        for j in range(NQC):
            if j + 1 < NQC:
                queue_proj(j + 1)
            jq = slice(j * 512, (j + 1) * 512)
            nkt = 4 * (j + 1)
            # transposed denominators: dT[r, lt, ph] = den[ph][lt*128+r]
            dT = nrm.tile([128, 8, 4], BF16, tag="dT")
            rT = nrm.tile([128, 8, 4], F32, tag="rT")
            for p in range(NP):
                pop_work(3)
                oA = pso.tile([65, 512], F32, tag="oA")
                oB = pso.tile([65, 512], F32, tag="oB")
                pts = {}

                def attn_v(kt):
                    i = kt - 4 * j
                    span = 512 if i < 0 else 512 - 128 * i
                    co = 512 - span
                    pt = pts.pop(kt)
                    for h, o in ((0, oA), (1, oB)):
                        nc.tensor.matmul(
                            o[:, co:512],
                            vp_sb[:, kt, p * 2 + h, :],
                            pt[:, h, 0:span],
                            start=(kt == 0),
                            stop=(kt == nkt - 1),
                        )

                for kt in range(nkt):
                    i = kt - 4 * j
                    span = 512 if i < 0 else 512 - 128 * i
                    q0 = j * 512 + (512 - span)
                    sc = pss.tile([128, 2, 512], F32, tag="sc")
                    for h in range(2):
                        nc.tensor.matmul(
                            sc[:, h, 0:span],
                            kT_sb[
                                h * 64 : (h + 1) * 64,
                                p,
                                kt * 128 : (kt + 1) * 128,
                            ],
                            qT_sb[h * 64 : (h + 1) * 64, p, q0 : q0 + span],
                            start=True,
                            stop=True,
                            tile_position=(h * 64, 0),
                        )
                    pt = ptp.tile([128, 2, 512], BF16, tag="pt")
                    pts[kt] = pt
                    nc.scalar.activation(pt[:, :, 0:span], sc[:, :, 0:span], AF.Exp)
                    if i >= 0:
                        # diagonal block: zero masked probs (post-exp, SBUF;
                        # a DVE RMW on the scores psum raced with the PE drain)
                        nc.vector.tensor_mul(
                            pt[:, :, 0:128], pt[:, :, 0:128], mask_sb[:]
                        )
                    if kt >= 1:
                        attn_v(kt - 1)
                attn_v(nkt - 1)

                # evict unnormalized outputs; reciprocal of the ones-column
                # denominators via a DMA-transposed layout (free-size-bound)
                ocs = []
                for h, o in ((0, oA), (1, oB)):
                    oc = ocp.tile([65, 512], BF16, tag=f"oc{h}")
                    nc.scalar.activation(oc[:], o[:], AF.Identity)
                    nc.sync.dma_start(
                        out=dT[:, 2 * p + h, :],
                        in_=oc[64:65, :].rearrange("o (r lt) -> o r lt", lt=4),
                    )
                    ocs.append(oc)
                nc.vector.reciprocal(
                    rT[:, 2 * p : 2 * p + 2, :], dT[:, 2 * p : 2 * p + 2, :]
                )
                for h, oc in ((0, ocs[0]), (1, ocs[1])):
                    rr1 = nrm.tile([1, 512], F32, tag=f"rr{h}")
                    nc.sync.dma_start(
                        out=rr1.rearrange("o (r lt) -> o r lt", lt=4),
                        in_=rT[:, 2 * p + h, :],
                    )
                    rb = nrm.tile([64, 512], F32, tag=f"rb{h}")
                    nc.gpsimd.partition_broadcast(rb[:], rr1[:])
                    nc.vector.tensor_mul(
                        OT_sb[h * 64 : (h + 1) * 64, p, jq], oc[0:64, :], rb[:]
                    )

            for tt in range(j * 4, (j + 1) * 4):
                work_q.append(out_proj(tt))
        pop_work(len(work_q))

    nc.compile()
    return nc


def _rope_tables():
    freqs = 1.0 / (ROPE_THETA ** (np.arange(0, HD, 2, dtype=np.float32) / HD))
    ang = np.arange(T, dtype=np.float32)[:, None] * freqs[None, :]  # [T, 32]
    cos = np.cos(ang).T  # [32, T]
    sin = np.sin(ang).T
    # interleaved-lane tables: lane 2i,2i+1 <- freq i; sin sign -,+
    cosI = np.empty((64, T), np.float32)
    sinI = np.empty((64, T), np.float32)
    cosI[0::2] = cos
    cosI[1::2] = cos
    sinI[0::2] = -sin
    sinI[1::2] = sin
    cos128 = np.tile(cosI, (2, 1))
    sin128 = np.tile(sinI, (2, 1))
    return cos128, sin128


def _bf16(a):
    return np.asarray(a, np.float32).astype(ml_dtypes.bfloat16)


def _prep_inputs(x, w_qkv, b_qkv, w_proj, b_proj):
    cos128, sin128 = _rope_tables()
    km = np.arange(128)
    mneg_np = np.where(km[:, None] <= km[None, :], 1.0, 0.0).astype(np.float32)
    # per-head channel interleave: new dim 2i <- i, 2i+1 <- 32+i
    perm1 = np.empty(64, np.int64)
    perm1[0::2] = np.arange(32)
    perm1[1::2] = np.arange(32, 64)
    perm = np.concatenate([perm1 + 64 * h for h in range(HH)])
    in_maps = []
    for c in range(NCORE):
        b, hh = c // 2, c % 2
        s = hh * 512
        m = {
            "xT": _bf16(x[b].T),
            "wq": _bf16(w_qkv[:, s : s + 512][:, perm] / 8.0),
            "wk": _bf16(w_qkv[:, C + s : C + s + 512][:, perm]),
            "wv": _bf16(w_qkv[:, 2 * C + s : 2 * C + s + 512]),
            "wp": _bf16(w_proj[s : s + 512, :]),
            "bqk": np.stack(
                [
                    (b_qkv[s : s + 512][perm] / 8.0).reshape(NP, 128),
                    b_qkv[C + s : C + s + 512][perm].reshape(NP, 128),
                ]
            ).astype(np.float32),
            "cosr": _bf16(cos128),
            "sinp": _bf16(sin128),
            "mneg": _bf16(np.stack([mneg_np, mneg_np], axis=1)),
        }
        in_maps.append(m)
    return in_maps


def _run(x, w_qkv, b_qkv, w_proj, b_proj, trace=False):
    if "nc" not in _CACHE:
        _CACHE["nc"] = _build_module()
    nc = _CACHE["nc"]
    x = np.asarray(x, np.float32)
    w_qkv = np.asarray(w_qkv, np.float32)
    b_qkv = np.asarray(b_qkv, np.float32)
    w_proj = np.asarray(w_proj, np.float32)
    b_proj = np.asarray(b_proj, np.float32)
    in_maps = _prep_inputs(x, w_qkv, b_qkv, w_proj, b_proj)
    res = run_bass_kernel_spmd(nc, in_maps, core_ids=list(range(NCORE)), trace=trace)
    # v-bias commutes through the softmax average: its contribution to y is
    # the constant row bv @ w_proj, added here along with b_proj
    bias = b_proj + b_qkv[2 * C :] @ w_proj
    out = np.empty((B, T, C), np.float32)
    for b in range(B):
        out[b] = res.results[2 * b]["y"] + res.results[2 * b + 1]["y"] + bias
    return out, res


def kernel(x, w_qkv, b_qkv, w_proj, b_proj, n_heads=16):
    out, _ = _run(x, w_qkv, b_qkv, w_proj, b_proj, trace=False)
    return out


# revision 44
# speedup vs baseline: 1.0317x; 1.0317x over previous
"""Causal self-attention (B=4, T=2048, C=1024, H=16) on 8 TRN2 NeuronCores.

Sharding: core c handles batch b=c//2 and head-half hh=c%2 (8 heads).
Each core computes q/k/v projections for its heads, causal attention, and a
partial output projection (row-parallel w_proj); the host sums the two
partials per batch and adds b_proj plus the v-bias term (bv @ wp commutes
through the softmax average, so it is a host-side constant).

Layout: transposed attention (S^T = K Q^T with keys on psum partitions) so
softmax needs no transposes; a ones column in V yields softmax denominators
from the attn@v matmul. All matmul operands are bf16 (fp32 psum accumulate).
RoPE channel pairs are interleaved on partitions host-side so rotate-half is
a single DVE stream_shuffle (scores are invariant to the permutation).
x^T stays resident in SBUF. q/k/out projections run as K=64 row-group pairs
at tile_position (0,0)/(64,0) so the two streams overlap and weight loads
hide under the concurrent matmul. Denominator reciprocals run on a
DMA-transposed [128, chunk] layout (free-size-bound DVE op). Projections,
attention, and the output projection are software-pipelined per 512-query
chunk via a deferred work queue.
"""

import sys

sys.path.insert(0, "/opt/trn_rl_repo")

from contextlib import ExitStack

import numpy as np
import ml_dtypes

import concourse.bass as bass
import concourse.tile as tile
from concourse import bacc, mybir
from concourse.bass_utils import run_bass_kernel_spmd

F32 = mybir.dt.float32
BF16 = mybir.dt.bfloat16
AL = mybir.AluOpType
AF = mybir.ActivationFunctionType

B, T, C, H, HD = 4, 2048, 1024, 16, 64
NCORE = 8
HH = H // 2  # heads per core (8)
NP = HH // 2  # head pairs per core (4)
KC = C // 128  # contraction chunks (8)
NT = T // 128  # 128-row time tiles (16)
NQC = T // 512  # 512-query chunks (4)
ROPE_THETA = 10000.0
SHUF_SWAP = [l ^ 1 for l in range(32)]

_CACHE = {}


def _build_module():
    nc = bacc.Bacc("TRN2", target_bir_lowering=False, debug=False)

    xT = nc.dram_tensor("xT", [C, T], BF16, kind="ExternalInput")
    wq = nc.dram_tensor("wq", [C, 512], BF16, kind="ExternalInput")
    wk = nc.dram_tensor("wk", [C, 512], BF16, kind="ExternalInput")
    wv = nc.dram_tensor("wv", [C, 512], BF16, kind="ExternalInput")
    wp = nc.dram_tensor("wp", [512, C], BF16, kind="ExternalInput")
    bqk = nc.dram_tensor("bqk", [2, NP, 128], F32, kind="ExternalInput")
    cosr = nc.dram_tensor("cosr", [128, T], BF16, kind="ExternalInput")
    sinp = nc.dram_tensor("sinp", [128, T], BF16, kind="ExternalInput")
    mneg = nc.dram_tensor("mneg", [128, 2, 128], BF16, kind="ExternalInput")
    y = nc.dram_tensor("y", [T, C], F32, kind="ExternalOutput")

    with tile.TileContext(nc) as tc, ExitStack() as ctx:
        consts = ctx.enter_context(tc.tile_pool(name="consts", bufs=1))
        bigp = ctx.enter_context(tc.tile_pool(name="big", bufs=1))
        stg = ctx.enter_context(tc.tile_pool(name="stg", bufs=4))
        ptp = ctx.enter_context(tc.tile_pool(name="ptp", bufs=6))
        ocp = ctx.enter_context(tc.tile_pool(name="ocp", bufs=4))
        nrm = ctx.enter_context(tc.tile_pool(name="nrm", bufs=2))
        ysp = ctx.enter_context(tc.tile_pool(name="ysp", bufs=3))
        psg = ctx.enter_context(tc.tile_pool(name="psg", bufs=2, space="PSUM"))
        pss = ctx.enter_context(tc.tile_pool(name="pss", bufs=2, space="PSUM"))
        pso = ctx.enter_context(tc.tile_pool(name="pso", bufs=1, space="PSUM"))

        # ---- constants ----
        bqk_sb = consts.tile([128, 2, NP], F32)
        nc.sync.dma_start(out=bqk_sb[:], in_=bqk.rearrange("a p r -> r a p"))
        cos_sb = consts.tile([128, T], BF16)
        sin_sb = consts.tile([128, T], BF16)
        nc.sync.dma_start(out=cos_sb[:], in_=cosr[:])
        nc.sync.dma_start(out=sin_sb[:], in_=sinp[:])
        mask_sb = consts.tile([128, 2, 128], BF16)
        nc.sync.dma_start(out=mask_sb[:], in_=mneg[:])

        wq_sb = consts.tile([128, KC, 512], BF16)
        wk_sb = consts.tile([128, KC, 512], BF16)
        wv_sb = consts.tile([128, KC, 512], BF16)
        for wt, wsb in ((wq, wq_sb), (wk, wk_sb), (wv, wv_sb)):
            nc.sync.dma_start(
                out=wsb[:], in_=wt.rearrange("(kc p) m -> p kc m", p=128)
            )
        wp_sb = consts.tile([128, NP, C], BF16)
        nc.sync.dma_start(
            out=wp_sb[:], in_=wp.rearrange("(kc r) n -> r kc n", r=128)
        )

        # resident x^T, loaded once in 4 chunks so chunk-0 work starts early
        xT_sb = bigp.tile([128, KC, T], BF16)
        xTr = xT.rearrange("(kc p) t -> p kc t", p=128)
        for j in range(NQC):
            nk = slice(j * 512, (j + 1) * 512)
            nc.sync.dma_start(out=xT_sb[:, :, nk], in_=xTr[:, :, nk])

        qT_sb = bigp.tile([128, NP, T], BF16)
        kT_sb = bigp.tile([128, NP, T], BF16)
        vp_sb = bigp.tile([128, NT, HH, 65], BF16)
        OT_sb = bigp.tile([128, NP, T], BF16)
        nc.vector.memset(vp_sb[:, :, :, 64:65], 1.0)

        # ---- work units (closures), interleaved via a deferred queue ----
        def proj_v(tt):
            def go():
                vps = psg.tile([128, 512], F32, tag="g")
                for kc in range(KC):
                    nc.tensor.matmul(
                        vps[:],
                        xT_sb[:, kc, tt * 128 : (tt + 1) * 128],
                        wv_sb[:, kc, :],
                        start=(kc == 0),
                        stop=(kc == KC - 1),
                    )
                nc.vector.tensor_copy(
                    vp_sb[:, tt, :, 0:64],
                    vps.rearrange("p (h d) -> p h d", h=HH),
                )

            return go

        def proj_qk(which, wsb, qkout, nq, p):
            def go():
                nk = slice(nq * 512, (nq + 1) * 512)
                qps = psg.tile([128, 512], F32, tag="g")
                for kc in range(KC):
                    nc.tensor.matmul(
                        qps[:],
                        wsb[:, kc, p * 128 : (p + 1) * 128],
                        xT_sb[:, kc, nk],
                        start=(kc == 0),
                        stop=(kc == KC - 1),
                    )
                # evict psum + per-partition bias on ScalarE (ACT is idle
                # outside exp; keeps DVE free for attention evictions)
                qsf = stg.tile([128, 512], BF16, tag="qsf")
                nc.scalar.activation(
                    qsf[:], qps[:], AF.Identity, bias=bqk_sb[:, which, p : p + 1]
                )
                qsh = stg.tile([128, 512], BF16, tag="qsh")
                nc.vector.stream_shuffle(qsh[:], qsf[:], SHUF_SWAP)
                m1 = stg.tile([128, 512], BF16, tag="m1")
                nc.vector.tensor_mul(m1[:], qsf[:], cos_sb[:, nk])
                m2 = stg.tile([128, 512], BF16, tag="m2")
                nc.vector.tensor_mul(m2[:], qsh[:], sin_sb[:, nk])
                nc.vector.tensor_add(qkout[:, p, nk], m1[:], m2[:])

            return go

        def out_proj(tt):
            def go():
                ysb = ysp.tile([128, C], F32, tag="ysb")
                for nn in range(2):
                    yps = psg.tile([128, 512], F32, tag="g")
                    for kc in range(NP):
                        nc.tensor.matmul(
                            yps[:],
                            OT_sb[:, kc, tt * 128 : (tt + 1) * 128],
                            wp_sb[:, kc, nn * 512 : (nn + 1) * 512],
                            start=(kc == 0),
                            stop=(kc == NP - 1),
                        )
                    nc.vector.tensor_copy(ysb[:, nn * 512 : (nn + 1) * 512], yps[:])
                nc.sync.dma_start(out=y[tt * 128 : (tt + 1) * 128, :], in_=ysb[:])

            return go

        work_q = []

        def pop_work(n):
            for _ in range(min(n, len(work_q))):
                work_q.pop(0)()

        def queue_proj(j):
            for tt in range(j * 4, (j + 1) * 4):
                work_q.append(proj_v(tt))
            for p in range(NP):
                work_q.append(proj_qk(0, wq_sb, qT_sb, j, p))
                work_q.append(proj_qk(1, wk_sb, kT_sb, j, p))

        # chunk-0 projections must run before attention j=0
        queue_proj(0)
        pop_work(len(work_q))

        for j in range(NQC):
            if j + 1 < NQC:
                queue_proj(j + 1)
            jq = slice(j * 512, (j + 1) * 512)
            nkt = 4 * (j + 1)
            # transposed denominators: dT[r, lt, ph] = den[ph][lt*128+r]
            dT = nrm.tile([128, 8, 4], BF16, tag="dT")
            rT = nrm.tile([128, 8, 4], F32, tag="rT")
            for p in range(NP):
                pop_work(3)
                oA = pso.tile([65, 512], F32, tag="oA")
                oB = pso.tile([65, 512], F32, tag="oB")
                pts = {}

                def attn_v(kt):
                    i = kt - 4 * j
                    span = 512 if i < 0 else 512 - 128 * i
                    co = 512 - span
                    pt = pts.pop(kt)
                    for h, o in ((0, oA), (1, oB)):
                        nc.tensor.matmul(
                            o[:, co:512],
                            vp_sb[:, kt, p * 2 + h, :],
                            pt[:, h, 0:span],
                            start=(kt == 0),
                            stop=(kt == nkt - 1),
                        )

                for kt in range(nkt):
                    i = kt - 4 * j
                    span = 512 if i < 0 else 512 - 128 * i
                    q0 = j * 512 + (512 - span)
                    sc = pss.tile([128, 2, 512], F32, tag="sc")
                    for h in range(2):
                        nc.tensor.matmul(
                            sc[:, h, 0:span],
                            kT_sb[
                                h * 64 : (h + 1) * 64,
                                p,
                                kt * 128 : (kt + 1) * 128,
                            ],
                            qT_sb[h * 64 : (h + 1) * 64, p, q0 : q0 + span],
                            start=True,
                            stop=True,
                            tile_position=(h * 64, 0),
                        )
                    pt = ptp.tile([128, 2, 512], BF16, tag="pt")
                    pts[kt] = pt
                    nc.scalar.activation(pt[:, :, 0:span], sc[:, :, 0:span], AF.Exp)
                    if i >= 0:
                        # diagonal block: zero masked probs (post-exp, SBUF;
                        # a DVE RMW on the scores psum raced with the PE drain)
                        nc.vector.tensor_mul(
                            pt[:, :, 0:128], pt[:, :, 0:128], mask_sb[:]
                        )
                    if kt >= 1:
                        attn_v(kt - 1)
                attn_v(nkt - 1)

                # evict unnormalized outputs; reciprocal of the ones-column
                # denominators via a DMA-transposed layout (free-size-bound)
                ocs = []
                for h, o in ((0, oA), (1, oB)):
                    oc = ocp.tile([65, 512], BF16, tag=f"oc{h}")
                    nc.scalar.activation(oc[:], o[:], AF.Identity)
                    nc.sync.dma_start(
                        out=dT[:, 2 * p + h, :],
                        in_=oc[64:65, :].rearrange("o (r lt) -> o r lt", lt=4),
                    )
                    ocs.append(oc)
                nc.vector.reciprocal(
                    rT[:, 2 * p : 2 * p + 2, :], dT[:, 2 * p : 2 * p + 2, :]
                )
                for h, oc in ((0, ocs[0]), (1, ocs[1])):
                    rr1 = nrm.tile([1, 512], F32, tag=f"rr{h}")
                    nc.sync.dma_start(
                        out=rr1.rearrange("o (r lt) -> o r lt", lt=4),
                        in_=rT[:, 2 * p + h, :],
                    )
                    rb = nrm.tile([64, 512], F32, tag=f"rb{h}")
                    nc.gpsimd.partition_broadcast(rb[:], rr1[:])
                    nc.vector.tensor_mul(
                        OT_sb[h * 64 : (h + 1) * 64, p, jq], oc[0:64, :], rb[:]
                    )

            for tt in range(j * 4, (j + 1) * 4):
                work_q.append(out_proj(tt))
        pop_work(len(work_q))

    nc.compile()
    return nc


def _rope_tables():
    freqs = 1.0 / (ROPE_THETA ** (np.arange(0, HD, 2, dtype=np.float32) / HD))
    ang = np.arange(T, dtype=np.float32)[:, None] * freqs[None, :]  # [T, 32]
    cos = np.cos(ang).T  # [32, T]
    sin = np.sin(ang).T
    # interleaved-lane tables: lane 2i,2i+1 <- freq i; sin sign -,+
    cosI = np.empty((64, T), np.float32)
    sinI = np.empty((64, T), np.float32)
    cosI[0::2] = cos
    cosI[1::2] = cos
    sinI[0::2] = -sin
    sinI[1::2] = sin
    cos128 = np.tile(cosI, (2, 1))
    sin128 = np.tile(sinI, (2, 1))
    return cos128, sin128


def _bf16(a):
    return np.asarray(a, np.float32).astype(ml_dtypes.bfloat16)


def _prep_inputs(x, w_qkv, b_qkv, w_proj, b_proj):
    cos128, sin128 = _rope_tables()
    km = np.arange(128)
    mneg_np = np.where(km[:, None] <= km[None, :], 1.0, 0.0).astype(np.float32)
    # per-head channel interleave: new dim 2i <- i, 2i+1 <- 32+i
    perm1 = np.empty(64, np.int64)
    perm1[0::2] = np.arange(32)
    perm1[1::2] = np.arange(32, 64)
    perm = np.concatenate([perm1 + 64 * h for h in range(HH)])
    in_maps = []
    for c in range(NCORE):
        b, hh = c // 2, c % 2
        s = hh * 512
        m = {
            "xT": _bf16(x[b].T),
            "wq": _bf16(w_qkv[:, s : s + 512][:, perm] / 8.0),
            "wk": _bf16(w_qkv[:, C + s : C + s + 512][:, perm]),
            "wv": _bf16(w_qkv[:, 2 * C + s : 2 * C + s + 512]),
            "wp": _bf16(w_proj[s : s + 512, :]),
            "bqk": np.stack(
                [
                    (b_qkv[s : s + 512][perm] / 8.0).reshape(NP, 128),
                    b_qkv[C + s : C + s + 512][perm].reshape(NP, 128),
                ]
            ).astype(np.float32),
            "cosr": _bf16(cos128),
            "sinp": _bf16(sin128),
            "mneg": _bf16(np.stack([mneg_np, mneg_np], axis=1)),
        }
        in_maps.append(m)
    return in_maps


def _run(x, w_qkv, b_qkv, w_proj, b_proj, trace=False):
    if "nc" not in _CACHE:
        _CACHE["nc"] = _build_module()
    nc = _CACHE["nc"]
    x = np.asarray(x, np.float32)
    w_qkv = np.asarray(w_qkv, np.float32)
    b_qkv = np.asarray(b_qkv, np.float32)
    w_proj = np.asarray(w_proj, np.float32)
    b_proj = np.asarray(b_proj, np.float32)
    in_maps = _prep_inputs(x, w_qkv, b_qkv, w_proj, b_proj)
    res = run_bass_kernel_spmd(nc, in_maps, core_ids=list(range(NCORE)), trace=trace)
    # v-bias commutes through the softmax average: its contribution to y is
    # the constant row bv @ w_proj, added here along with b_proj
    bias = b_proj + b_qkv[2 * C :] @ w_proj
    out = np.empty((B, T, C), np.float32)
    for b in range(B):
        out[b] = res.results[2 * b]["y"] + res.results[2 * b + 1]["y"] + bias
    return out, res


def kernel(x, w_qkv, b_qkv, w_proj, b_proj, n_heads=16):
    out, _ = _run(x, w_qkv, b_qkv, w_proj, b_proj, trace=False)
    return out
